# revision 18
# baseline (speedup 1.0000x reference)
"""GQA (16 q-heads / 4 kv-heads, D=128, S=2048, E=2048, B=2) on 8 trn2 cores.

Sharding: core = 4*b + g  (b in {0,1} batch, g in {0..3} kv-head group).
Each core computes its batch's 4 query heads (one kv group) end-to-end and
the host sums the 4 partial o_proj outputs per batch.

v3 (single interleaved program, engines balanced, DMA-consolidated):
  - Blocked DRAM layouts + AP.transpose give one DMA per weight tensor and
    4 DMAs per x position-chunk (~65 DMAs total vs ~250).
  - Phase A per 512-position chunk: K proj+RoPE, Q(h0) proj+RoPE, V proj +
    PE-transpose to natural bf16, Q(h1..h3) proj+RoPE.  RoPE rotate-half is
    a signed-permutation matmul on PE (no SBUF swap DMAs); cos/sin mults
    split DVE/DVE, add on Pool.
  - Attention in four 512-wide query chunks: scoresT per sk-tile in f32r,
    Exp on Act straight from PSUM to bf16 SBUF, bf16 AV matmuls (V-natural
    stationary), softmax denominator via two bf16 chain-adds (DVE + Pool),
    ones-matmul partition reduce, reciprocal, Pool partition_broadcast,
    DVE normalize multiply into f32r ot.
  - o_proj of chunk c interleaved into attention of chunk c+1; stores
    staged through SBUF [128,1024] tiles via DVE/Act copies.
"""

import numpy as np
import ml_dtypes

import concourse.bass as bass
import concourse.bacc as bacc
import concourse.mybir as mybir
import concourse.tile as tile
from concourse.bass_utils import run_bass_kernel_spmd

B, S, E = 2, 2048, 2048
H, HKV, D = 16, 4, 128
G = H // HKV          # 4 query heads per kv group
GD = G * D            # 512 channels per group
NCORES = 8
SCALE = 1.0 / float(np.sqrt(D))
ROPE_BASE = 10000.0

NE = E // 128         # 16 e-blocks (contraction for projections)
NC4 = S // 512        # 4 position chunks of 512 (projection granularity)
NST = S // 128        # 16 sk-tiles of 128
CH = 512              # attention query-chunk width
NCH = S // CH         # 4 attention chunks

F32 = mybir.dt.float32
F32R = mybir.dt.float32r
BF16 = mybir.dt.bfloat16
AF = mybir.ActivationFunctionType
OP = mybir.AluOpType


def _r(ap):
    return ap.bitcast(F32R)


def _emit(nc, tc, xTb, wqb, wkb, wvb, wob, cosT, sinT, rotP, ident, onesb, out):
    from contextlib import ExitStack
    es = ExitStack()
    with es:
        cpool = es.enter_context(tc.tile_pool(name="const", bufs=1))
        wopool = es.enter_context(tc.tile_pool(name="wo", bufs=2))
        xpool = es.enter_context(tc.tile_pool(name="xs", bufs=5))
        rpool = es.enter_context(tc.tile_pool(name="rope", bufs=2))
        etpool = es.enter_context(tc.tile_pool(name="et", bufs=6))
        bcspool = es.enter_context(tc.tile_pool(name="bcs", bufs=2))
        dnpool = es.enter_context(tc.tile_pool(name="dn", bufs=2))
        rcpool = es.enter_context(tc.tile_pool(name="rc", bufs=2))
        otpool = es.enter_context(tc.tile_pool(name="ot", bufs=6))
        ostgpool = es.enter_context(tc.tile_pool(name="ostg", bufs=2))
        vtpool = es.enter_context(tc.tile_pool(name="vt", bufs=2))
        pssc = es.enter_context(
            tc.tile_pool(name="pssc", bufs=2, space=bass.MemorySpace.PSUM))
        psav = es.enter_context(
            tc.tile_pool(name="psav", bufs=1, space=bass.MemorySpace.PSUM))
        psmx = es.enter_context(
            tc.tile_pool(name="psmx", bufs=4, space=bass.MemorySpace.PSUM))

        # ---- small constants ----
        id_sb = cpool.tile([128, 128], F32, tag="id")
        ones_sb = cpool.tile([128, 128], BF16, tag="ones")
        rp_sb = cpool.tile([128, 128], F32R, tag="rp")
        nc.sync.dma_start(out=id_sb[:], in_=ident.ap())
        nc.sync.dma_start(out=ones_sb[:], in_=onesb.ap())
        nc.sync.dma_start(out=rp_sb[:], in_=rotP.ap().bitcast(F32R))
        cos_sb = cpool.tile([D, S], BF16, tag="cos")
        sin_sb = cpool.tile([D, S], BF16, tag="sin")
        nc.sync.dma_start(out=cos_sb[:], in_=cosT.ap())
        nc.sync.dma_start(out=sin_sb[:], in_=sinT.ap())

        # ---- consolidated weight loads (one DMA each) ----
        wk_t = cpool.tile([128, NE, D], F32R, tag="wkt")
        nc.sync.dma_start(out=wk_t[:],
                          in_=wkb.ap().transpose([1, 0, 2]).bitcast(F32R))
        wq_t = cpool.tile([128, NE, GD], F32R, tag="wqt")
        nc.sync.dma_start(out=wq_t[:],
                          in_=wqb.ap().transpose([1, 0, 2]).bitcast(F32R))
        wv_t = cpool.tile([128, NE, D], F32R, tag="wvt")
        nc.sync.dma_start(out=wv_t[:],
                          in_=wvb.ap().transpose([1, 0, 2]).bitcast(F32R))

        # ---- persistent activations ----
        kt = cpool.tile([D, S], F32R, tag="kt")
        qt = [cpool.tile([D, S], F32R, tag=f"qt{h}", name=f"qt{h}")
              for h in range(G)]
        vn = cpool.tile([128, NST, D], BF16, tag="vn")

        def rope(dst, ps, sl):
            # rotate_half as a signed-permutation matmul; then
            # dst = q*cos + rot(q)*sin.
            qraw = rpool.tile([128, 512], F32R, tag="qraw")
            tmc = rpool.tile([128, 512], F32, tag="tmc")
            t2 = rpool.tile([128, 512], F32, tag="t2")
            nc.scalar.copy(qraw[:], ps[:])
            rot = psmx.tile([128, 512], F32, tag="mx", name="rot")
            nc.tensor.matmul(rot[:], rp_sb[:], qraw[:], start=True, stop=True)
            nc.vector.tensor_tensor(tmc[:], qraw[:], cos_sb[:, sl], OP.mult)
            nc.vector.tensor_tensor(t2[:], rot[:], sin_sb[:, sl], OP.mult)
            nc.gpsimd.tensor_tensor(dst, tmc[:], t2[:], OP.add)

        def proj(wt, cslice, xsl, c4):
            ps = psmx.tile([128, 512], F32, tag="mx", name="ps")
            for j in range(NE):
                nc.tensor.matmul(ps[:], wt[:, j, cslice],
                                 xsl[j // 4][:, j % 4, :],
                                 start=(j == 0), stop=(j == NE - 1))
            return ps

        # ================= phase A: projections + RoPE =================
        for c4 in range(NC4):
            sl = slice(c4 * 512, (c4 + 1) * 512)
            xsl = [xpool.tile([128, 4, 512], F32R, tag="xs",
                              name=f"xs{c4}_{jj}") for jj in range(4)]
            for jj in range(4):
                nc.sync.dma_start(
                    out=xsl[jj][:],
                    in_=xTb.ap()[jj * 4:(jj + 1) * 4, :, sl]
                        .transpose([1, 0, 2]).bitcast(F32R))
            # K
            ps = proj(wk_t, slice(0, D), xsl, c4)
            rope(kt[:, sl], ps, sl)
            # Q head 0 early so attention can start right after phase A
            ps = proj(wq_t, slice(0, D), xsl, c4)
            rope(qt[0][:, sl], ps, sl)
            # V -> natural bf16 via PE transpose
            ps = proj(wv_t, slice(0, D), xsl, c4)
            vt = vtpool.tile([D, 512], F32, tag="vt")
            nc.scalar.copy(vt[:], ps[:])
            for tt in range(4):
                trp = psmx.tile([128, 128], F32, tag="mx", name="trp")
                nc.tensor.transpose(trp[:], vt[:, tt * 128:(tt + 1) * 128],
                                    id_sb[:])
                nc.vector.tensor_copy(vn[:, c4 * 4 + tt, :], trp[:])
            # Q heads 1..3
            for h in range(1, G):
                ps = proj(wq_t, slice(h * D, (h + 1) * D), xsl, c4)
                rope(qt[h][:, sl], ps, sl)

        # ================= phases B+C: attention + o_proj =================
        def attn_head(c, h):
            """Scores/exp/AV/denominator/normalize for (chunk c, head h).
            Returns the normalized ot tile [D, CH] (f32r)."""
            dnA = dnpool.tile([128, CH], BF16, tag="dnA")
            dnB = dnpool.tile([128, CH], BF16, tag="dnB")
            av = psav.tile([D, CH], F32, tag="av")
            sl = slice(c * CH, (c + 1) * CH)
            et_first = None
            for t in range(NST):
                sc = pssc.tile([128, CH], F32, tag="sc")
                nc.tensor.matmul(sc[:], kt[:, t * 128:(t + 1) * 128],
                                 qt[h][:, sl], start=True, stop=True)
                et = etpool.tile([128, CH], BF16, tag="et")
                nc.scalar.activation(et[:], sc[:], AF.Exp, scale=SCALE)
                nc.tensor.matmul(av[:], vn[:, t, :], et[:],
                                 start=(t == 0), stop=(t == NST - 1))
                # denominator: DVE chain for t0..10, Pool chain for t11..15
                if t == 0:
                    et_first = et
                elif t == 1:
                    nc.vector.tensor_tensor(dnA[:], et_first[:], et[:], OP.add)
                elif t <= 10:
                    nc.vector.tensor_tensor(dnA[:], dnA[:], et[:], OP.add)
                elif t == 11:
                    et_first = et
                elif t == 12:
                    nc.gpsimd.tensor_tensor(dnB[:], et_first[:], et[:], OP.add)
                else:
                    nc.gpsimd.tensor_tensor(dnB[:], dnB[:], et[:], OP.add)
            nc.vector.tensor_tensor(dnA[:], dnA[:], dnB[:], OP.add)
            sm = psmx.tile([1, CH], F32, tag="mx", name="sm")
            nc.tensor.matmul(sm[:], ones_sb[:, 0:1], dnA[:],
                             start=True, stop=True)
            rc = rcpool.tile([1, CH], BF16, tag="rc")
            with nc.allow_low_precision(reason="bf16 softmax denom recip"):
                nc.vector.reciprocal(rc[:], sm[:])
            bcs = bcspool.tile([128, CH], BF16, tag="bcs")
            nc.gpsimd.partition_broadcast(bcs[:], rc[:])
            ot = otpool.tile([D, CH], F32R, tag="ot")
            nc.vector.tensor_tensor(ot[:], av[:], bcs[:], OP.mult)
            return ot

        def oproj_pair(c, eo0, ots):
            """Two adjacent eo column-groups (1024 cols of E) of o_proj for
            chunk c: fill one [128,1024] staging tile per s-tile, one store."""
            wots = []
            for eo in (eo0, eo0 + 1):
                wot = wopool.tile([128, G, 512], F32R, tag="wo",
                                  name=f"wo{c}_{eo}")
                nc.sync.dma_start(
                    out=wot[:],
                    in_=wob.ap()[:, :, eo * 512:(eo + 1) * 512]
                        .transpose([1, 0, 2]).bitcast(F32R))
                wots.append(wot)
            for st in range(CH // 128):
                ostg = ostgpool.tile([128, 1024], F32, tag="ostg",
                                     name=f"ostg{c}_{eo0}_{st}")
                for k in range(2):
                    op = psmx.tile([128, 512], F32, tag="mx", name="op")
                    for h in range(G):
                        nc.tensor.matmul(op[:],
                                         ots[h][:, st * 128:(st + 1) * 128],
                                         wots[k][:, h, :],
                                         start=(h == 0), stop=(h == G - 1))
                    if k == 0:
                        nc.vector.tensor_copy(ostg[:, 0:512], op[:])
                    else:
                        nc.scalar.copy(ostg[:, 512:1024], op[:])
                nc.sync.dma_start(
                    out=out.ap()[c * CH + st * 128:c * CH + (st + 1) * 128,
                                 eo0 * 512:(eo0 + 2) * 512],
                    in_=ostg[:])

        prev_ots = None
        prev_c = -1
        for c in range(NCH):
            cur_ots = []
            for h in range(G):
                cur_ots.append(attn_head(c, h))
                # interleave previous chunk's o_proj; front-load (one eo-pair
                # after each of heads 0/1) so at most 4 prev + 2 cur ot tiles
                # are ever live.
                if prev_ots is not None and h < 2:
                    oproj_pair(prev_c, 2 * h, prev_ots)
            prev_ots = cur_ots
            prev_c = c
        oproj_pair(NCH - 1, 0, prev_ots)
        oproj_pair(NCH - 1, 2, prev_ots)


def _build():
    nc = bacc.Bacc("TRN2", target_bir_lowering=False, debug=False,
                   num_devices=NCORES)
    xTb = nc.dram_tensor("xTb", [NE, 128, S], F32, kind="ExternalInput")
    wqb = nc.dram_tensor("wqb", [NE, 128, GD], F32, kind="ExternalInput")
    wkb = nc.dram_tensor("wkb", [NE, 128, D], F32, kind="ExternalInput")
    wvb = nc.dram_tensor("wvb", [NE, 128, D], F32, kind="ExternalInput")
    wob = nc.dram_tensor("wob", [G, 128, E], F32, kind="ExternalInput")
    cosT = nc.dram_tensor("cosT", [D, S], BF16, kind="ExternalInput")
    sinT = nc.dram_tensor("sinT", [D, S], BF16, kind="ExternalInput")
    rotP = nc.dram_tensor("rotP", [128, 128], F32, kind="ExternalInput")
    ident = nc.dram_tensor("ident", [128, 128], F32, kind="ExternalInput")
    onesb = nc.dram_tensor("onesb", [128, 128], BF16, kind="ExternalInput")
    out = nc.dram_tensor("out", [S, E], F32, kind="ExternalOutput")
    with tile.TileContext(nc) as tc:
        _emit(nc, tc, xTb, wqb, wkb, wvb, wob, cosT, sinT, rotP, ident,
              onesb, out)
    nc.compile()
    return nc


def _rope_tables():
    inv = 1.0 / (ROPE_BASE ** (np.arange(0, D, 2, dtype=np.float64) / D))
    t = np.arange(S, dtype=np.float64)
    freqs = t[:, None] * inv[None, :]                    # [S, D/2]
    emb = np.concatenate([freqs, freqs], axis=-1)        # [S, D]
    cosT = np.cos(emb).T.astype(ml_dtypes.bfloat16)      # [D, S]
    sinT = np.sin(emb).T.astype(ml_dtypes.bfloat16)
    return np.ascontiguousarray(cosT), np.ascontiguousarray(sinT)


def _rot_perm():
    # rot(q)[d] = -q[d+64] for d<64, +q[d-64] for d>=64, as a stationary
    # matmul operand: rot = P^T @ q with P[k, m] below.
    p = np.zeros((128, 128), dtype=np.float32)
    for d in range(64):
        p[d + 64, d] = -1.0
        p[d, d + 64] = 1.0
    return p


_NC = None
LAST_RESULTS = None


def kernel(hidden_states, wq, wk, wv, wo):
    global _NC, LAST_RESULTS
    if _NC is None:
        _NC = _build()
    cosT, sinT = _rope_tables()
    ident = np.eye(128, dtype=np.float32)
    onesb = np.ones((128, 128), dtype=ml_dtypes.bfloat16)
    rotP = _rot_perm()
    hs = np.asarray(hidden_states, dtype=np.float32)
    wq = np.asarray(wq, dtype=np.float32)
    wk = np.asarray(wk, dtype=np.float32)
    wv = np.asarray(wv, dtype=np.float32)
    wo = np.asarray(wo, dtype=np.float32)

    in_maps = []
    for core in range(NCORES):
        b, g = divmod(core, G)
        in_maps.append({
            "xTb": np.ascontiguousarray(hs[b].T).reshape(NE, 128, S),
            "wqb": np.ascontiguousarray(
                wq[:, GD * g:GD * (g + 1)]).reshape(NE, 128, GD),
            "wkb": np.ascontiguousarray(
                wk[:, D * g:D * (g + 1)]).reshape(NE, 128, D),
            "wvb": np.ascontiguousarray(
                wv[:, D * g:D * (g + 1)]).reshape(NE, 128, D),
            "wob": np.ascontiguousarray(
                wo[GD * g:GD * (g + 1), :]).reshape(G, 128, E),
            "cosT": cosT,
            "sinT": sinT,
            "rotP": rotP,
            "ident": ident,
            "onesb": onesb,
        })

    res = run_bass_kernel_spmd(_NC, in_maps, list(range(NCORES)))
    LAST_RESULTS = res
    outs = [np.asarray(res.results[i]["out"], dtype=np.float32)
            for i in range(NCORES)]
    full = np.stack([sum(outs[b * G:(b + 1) * G]) for b in range(B)], axis=0)
    return full.astype(np.float32)


# revision 23
# speedup vs baseline: 1.1187x; 1.1187x over previous
"""GQA (16 q-heads / 4 kv-heads, D=128, S=2048, E=2048, B=2) on 8 trn2 cores.

Sharding: core = 4*b + g  (b in {0,1} batch, g in {0..3} kv-head group).
Each core computes its batch's 4 query heads (one kv group) end-to-end and
the host sums the 4 partial o_proj outputs per batch.

v3 (single interleaved program, engines balanced, DMA-consolidated):
  - Blocked DRAM layouts + AP.transpose give one DMA per weight tensor and
    4 DMAs per x position-chunk (~65 DMAs total vs ~250).
  - Phase A per 512-position chunk: K proj+RoPE, Q(h0) proj+RoPE, V proj +
    PE-transpose to natural bf16, Q(h1..h3) proj+RoPE.  RoPE rotate-half is
    a signed-permutation matmul on PE (no SBUF swap DMAs); cos/sin mults
    split DVE/DVE, add on Pool.
  - Attention in four 512-wide query chunks: scoresT per sk-tile in f32r,
    Exp on Act straight from PSUM to bf16 SBUF, bf16 AV matmuls (V-natural
    stationary), softmax denominator via two bf16 chain-adds (DVE + Pool),
    ones-matmul partition reduce, reciprocal, Pool partition_broadcast,
    DVE normalize multiply into f32r ot.
  - o_proj of chunk c interleaved into attention of chunk c+1; stores
    staged through SBUF [128,1024] tiles via DVE/Act copies.
"""

import numpy as np
import ml_dtypes

import concourse.bass as bass
import concourse.bacc as bacc
import concourse.mybir as mybir
import concourse.tile as tile
from concourse.bass_utils import run_bass_kernel_spmd

B, S, E = 2, 2048, 2048
H, HKV, D = 16, 4, 128
G = H // HKV          # 4 query heads per kv group
GD = G * D            # 512 channels per group
NCORES = 8
SCALE = 1.0 / float(np.sqrt(D))
ROPE_BASE = 10000.0

NE = E // 128         # 16 e-blocks (contraction for projections)
NC4 = S // 512        # 4 position chunks of 512 (projection granularity)
NST = S // 128        # 16 sk-tiles of 128
CH = 1024             # attention query-chunk width
NCH = S // CH         # 2 attention chunks

F32 = mybir.dt.float32
F32R = mybir.dt.float32r
BF16 = mybir.dt.bfloat16
AF = mybir.ActivationFunctionType
OP = mybir.AluOpType


def _r(ap):
    return ap.bitcast(F32R)


def _emit(nc, tc, xTb, wqb, wkb, wvb, wob, cosT, sinT, rotP, ident, onesb, out):
    from contextlib import ExitStack
    es = ExitStack()
    with es:
        cpool = es.enter_context(tc.tile_pool(name="const", bufs=1))
        wopool = es.enter_context(tc.tile_pool(name="wo", bufs=2))
        xpool = es.enter_context(tc.tile_pool(name="xs", bufs=4))
        rpool = es.enter_context(tc.tile_pool(name="rope", bufs=2))
        etpool = es.enter_context(tc.tile_pool(name="et", bufs=6))
        bcspool = es.enter_context(tc.tile_pool(name="bcs", bufs=1))
        dnpool = es.enter_context(tc.tile_pool(name="dn", bufs=2))
        rcpool = es.enter_context(tc.tile_pool(name="rc", bufs=2))
        otpool = es.enter_context(tc.tile_pool(name="ot", bufs=6))
        ostgpool = es.enter_context(tc.tile_pool(name="ostg", bufs=2))
        vtpool = es.enter_context(tc.tile_pool(name="vt", bufs=2))
        pssc = es.enter_context(
            tc.tile_pool(name="pssc", bufs=2, space=bass.MemorySpace.PSUM))
        psav = es.enter_context(
            tc.tile_pool(name="psav", bufs=1, space=bass.MemorySpace.PSUM))
        psmx = es.enter_context(
            tc.tile_pool(name="psmx", bufs=2, space=bass.MemorySpace.PSUM))

        # ---- small constants ----
        id_sb = cpool.tile([128, 128], F32, tag="id")
        ones_sb = cpool.tile([128, 128], BF16, tag="ones")
        rp_sb = cpool.tile([128, 128], F32R, tag="rp")
        nc.sync.dma_start(out=id_sb[:], in_=ident.ap())
        nc.sync.dma_start(out=ones_sb[:], in_=onesb.ap())
        nc.sync.dma_start(out=rp_sb[:], in_=rotP.ap().bitcast(F32R))
        cos_sb = cpool.tile([D, S], BF16, tag="cos")
        sin_sb = cpool.tile([D, S], BF16, tag="sin")
        nc.sync.dma_start(out=cos_sb[:], in_=cosT.ap())
        nc.sync.dma_start(out=sin_sb[:], in_=sinT.ap())

        # ---- consolidated weight loads (one DMA each) ----
        wk_t = cpool.tile([128, NE, D], F32R, tag="wkt")
        nc.sync.dma_start(out=wk_t[:],
                          in_=wkb.ap().transpose([1, 0, 2]).bitcast(F32R))
        wq_t = cpool.tile([128, NE, GD], F32R, tag="wqt")
        nc.sync.dma_start(out=wq_t[:],
                          in_=wqb.ap().transpose([1, 0, 2]).bitcast(F32R))
        wv_t = cpool.tile([128, NE, D], F32R, tag="wvt")
        nc.sync.dma_start(out=wv_t[:],
                          in_=wvb.ap().transpose([1, 0, 2]).bitcast(F32R))

        # ---- persistent activations (bf16: same PE rate, half SBUF) ----
        kt = cpool.tile([D, S], BF16, tag="kt")
        qt = [cpool.tile([D, S], BF16, tag=f"qt{h}", name=f"qt{h}")
              for h in range(G)]
        vn = cpool.tile([128, NST, D], BF16, tag="vn")

        def rope(dst, ps, sl):
            # rotate_half as a signed-permutation matmul; then
            # dst = q*cos + rot(q)*sin.
            qraw = rpool.tile([128, 512], F32R, tag="qraw")
            tmc = rpool.tile([128, 512], F32, tag="tmc")
            t2 = rpool.tile([128, 512], F32, tag="t2")
            nc.scalar.copy(qraw[:], ps[:])
            rot = psmx.tile([128, 512], F32, tag="mx", name="rot")
            nc.tensor.matmul(rot[:], rp_sb[:], qraw[:], start=True, stop=True)
            nc.vector.tensor_tensor(tmc[:], qraw[:], cos_sb[:, sl], OP.mult)
            nc.vector.tensor_tensor(t2[:], rot[:], sin_sb[:, sl], OP.mult)
            nc.gpsimd.tensor_tensor(dst, tmc[:], t2[:], OP.add)

        def proj(wt, cslice, xsl, c4):
            ps = psmx.tile([128, 512], F32, tag="mx", name="ps")
            for j in range(NE):
                nc.tensor.matmul(ps[:], wt[:, j, cslice],
                                 xsl[j // 4][:, j % 4, :],
                                 start=(j == 0), stop=(j == NE - 1))
            return ps

        # ================= phase A: projections + RoPE =================
        for c4 in range(NC4):
            sl = slice(c4 * 512, (c4 + 1) * 512)
            xsl = [xpool.tile([128, 4, 512], F32R, tag="xs",
                              name=f"xs{c4}_{jj}") for jj in range(4)]
            for jj in range(4):
                nc.sync.dma_start(
                    out=xsl[jj][:],
                    in_=xTb.ap()[jj * 4:(jj + 1) * 4, :, sl]
                        .transpose([1, 0, 2]).bitcast(F32R))
            # K
            ps = proj(wk_t, slice(0, D), xsl, c4)
            rope(kt[:, sl], ps, sl)
            # Q head 0 early so attention can start right after phase A
            ps = proj(wq_t, slice(0, D), xsl, c4)
            rope(qt[0][:, sl], ps, sl)
            # V -> natural bf16 via PE transpose
            ps = proj(wv_t, slice(0, D), xsl, c4)
            vt = vtpool.tile([D, 512], F32, tag="vt")
            nc.scalar.copy(vt[:], ps[:])
            for tt in range(4):
                trp = psmx.tile([128, 128], F32, tag="mx", name="trp")
                nc.tensor.transpose(trp[:], vt[:, tt * 128:(tt + 1) * 128],
                                    id_sb[:])
                nc.vector.tensor_copy(vn[:, c4 * 4 + tt, :], trp[:])
            # Q heads 1..3
            for h in range(1, G):
                ps = proj(wq_t, slice(h * D, (h + 1) * D), xsl, c4)
                rope(qt[h][:, sl], ps, sl)

        # ================= phases B+C: attention + o_proj =================
        def attn_head(c, h):
            """Scores/exp/AV/denominator/normalize for (chunk c, head h).
            Returns the normalized ot tile [D, CH] (f32r)."""
            dn = dnpool.tile([128, CH], BF16, tag="dn")
            av = psav.tile([D, CH], F32, tag="av")
            et_first = None
            for t in range(NST):
                sc = pssc.tile([128, CH], F32, tag="sc")
                for hf in range(2):
                    qsl = slice(c * CH + hf * 512, c * CH + (hf + 1) * 512)
                    nc.tensor.matmul(sc[:, hf * 512:(hf + 1) * 512],
                                     kt[:, t * 128:(t + 1) * 128],
                                     qt[h][:, qsl], start=True, stop=True)
                et = etpool.tile([128, CH], BF16, tag="et")
                nc.scalar.activation(et[:], sc[:], AF.Exp, scale=SCALE)
                for hf in range(2):
                    nc.tensor.matmul(av[:, hf * 512:(hf + 1) * 512],
                                     vn[:, t, :],
                                     et[:, hf * 512:(hf + 1) * 512],
                                     start=(t == 0), stop=(t == NST - 1))
                # denominator: single DVE chain, always caught up with exp
                if t == 0:
                    et_first = et
                elif t == 1:
                    nc.vector.tensor_tensor(dn[:], et_first[:], et[:], OP.add)
                else:
                    nc.vector.tensor_tensor(dn[:], dn[:], et[:], OP.add)
            rc = rcpool.tile([1, CH], BF16, tag="rc")
            for hf in range(2):
                sm = psmx.tile([1, 512], F32, tag="mx", name="sm")
                nc.tensor.matmul(sm[:], ones_sb[:, 0:1],
                                 dn[:, hf * 512:(hf + 1) * 512],
                                 start=True, stop=True)
                with nc.allow_low_precision(reason="bf16 softmax denom recip"):
                    nc.vector.reciprocal(rc[:, hf * 512:(hf + 1) * 512], sm[:])
            bcs = bcspool.tile([128, CH], BF16, tag="bcs")
            nc.gpsimd.partition_broadcast(bcs[:], rc[:])
            ot = otpool.tile([D, CH], F32R, tag="ot")
            nc.vector.tensor_tensor(ot[:], av[:], bcs[:], OP.mult)
            return ot

        def oproj_pair(c, eo0, ots):
            """Two adjacent eo column-groups (1024 cols of E) of o_proj for
            chunk c: fill one [128,1024] staging tile per s-tile, one store."""
            wots = []
            for eo in (eo0, eo0 + 1):
                wot = wopool.tile([128, G, 512], F32R, tag="wo",
                                  name=f"wo{c}_{eo}")
                nc.sync.dma_start(
                    out=wot[:],
                    in_=wob.ap()[:, :, eo * 512:(eo + 1) * 512]
                        .transpose([1, 0, 2]).bitcast(F32R))
                wots.append(wot)
            for st in range(CH // 128):
                ostg = ostgpool.tile([128, 1024], F32, tag="ostg",
                                     name=f"ostg{c}_{eo0}_{st}")
                for k in range(2):
                    op = psmx.tile([128, 512], F32, tag="mx", name="op")
                    for h in range(G):
                        nc.tensor.matmul(op[:],
                                         ots[h][:, st * 128:(st + 1) * 128],
                                         wots[k][:, h, :],
                                         start=(h == 0), stop=(h == G - 1))
                    nc.vector.tensor_copy(ostg[:, k * 512:(k + 1) * 512], op[:])
                nc.sync.dma_start(
                    out=out.ap()[c * CH + st * 128:c * CH + (st + 1) * 128,
                                 eo0 * 512:(eo0 + 2) * 512],
                    in_=ostg[:])

        prev_ots = None
        prev_c = -1
        for c in range(NCH):
            cur_ots = []
            for h in range(G):
                cur_ots.append(attn_head(c, h))
                # interleave previous chunk's o_proj; front-load (one eo-pair
                # after each of heads 0/1) so at most 4 prev + 2 cur ot tiles
                # are ever live.
                if prev_ots is not None and h < 2:
                    oproj_pair(prev_c, 2 * h, prev_ots)
            prev_ots = cur_ots
            prev_c = c
        oproj_pair(NCH - 1, 0, prev_ots)
        oproj_pair(NCH - 1, 2, prev_ots)


def _build():
    nc = bacc.Bacc("TRN2", target_bir_lowering=False, debug=False,
                   num_devices=NCORES)
    xTb = nc.dram_tensor("xTb", [NE, 128, S], F32, kind="ExternalInput")
    wqb = nc.dram_tensor("wqb", [NE, 128, GD], F32, kind="ExternalInput")
    wkb = nc.dram_tensor("wkb", [NE, 128, D], F32, kind="ExternalInput")
    wvb = nc.dram_tensor("wvb", [NE, 128, D], F32, kind="ExternalInput")
    wob = nc.dram_tensor("wob", [G, 128, E], F32, kind="ExternalInput")
    cosT = nc.dram_tensor("cosT", [D, S], BF16, kind="ExternalInput")
    sinT = nc.dram_tensor("sinT", [D, S], BF16, kind="ExternalInput")
    rotP = nc.dram_tensor("rotP", [128, 128], F32, kind="ExternalInput")
    ident = nc.dram_tensor("ident", [128, 128], F32, kind="ExternalInput")
    onesb = nc.dram_tensor("onesb", [128, 128], BF16, kind="ExternalInput")
    out = nc.dram_tensor("out", [S, E], F32, kind="ExternalOutput")
    with tile.TileContext(nc) as tc:
        _emit(nc, tc, xTb, wqb, wkb, wvb, wob, cosT, sinT, rotP, ident,
              onesb, out)
    nc.compile()
    return nc


def _rope_tables():
    inv = 1.0 / (ROPE_BASE ** (np.arange(0, D, 2, dtype=np.float64) / D))
    t = np.arange(S, dtype=np.float64)
    freqs = t[:, None] * inv[None, :]                    # [S, D/2]
    emb = np.concatenate([freqs, freqs], axis=-1)        # [S, D]
    cosT = np.cos(emb).T.astype(ml_dtypes.bfloat16)      # [D, S]
    sinT = np.sin(emb).T.astype(ml_dtypes.bfloat16)
    return np.ascontiguousarray(cosT), np.ascontiguousarray(sinT)


def _rot_perm():
    # rot(q)[d] = -q[d+64] for d<64, +q[d-64] for d>=64, as a stationary
    # matmul operand: rot = P^T @ q with P[k, m] below.
    p = np.zeros((128, 128), dtype=np.float32)
    for d in range(64):
        p[d + 64, d] = -1.0
        p[d, d + 64] = 1.0
    return p


_NC = None
LAST_RESULTS = None


def kernel(hidden_states, wq, wk, wv, wo):
    global _NC, LAST_RESULTS
    if _NC is None:
        _NC = _build()
    cosT, sinT = _rope_tables()
    ident = np.eye(128, dtype=np.float32)
    onesb = np.ones((128, 128), dtype=ml_dtypes.bfloat16)
    rotP = _rot_perm()
    hs = np.asarray(hidden_states, dtype=np.float32)
    wq = np.asarray(wq, dtype=np.float32)
    wk = np.asarray(wk, dtype=np.float32)
    wv = np.asarray(wv, dtype=np.float32)
    wo = np.asarray(wo, dtype=np.float32)

    in_maps = []
    for core in range(NCORES):
        b, g = divmod(core, G)
        in_maps.append({
            "xTb": np.ascontiguousarray(hs[b].T).reshape(NE, 128, S),
            "wqb": np.ascontiguousarray(
                wq[:, GD * g:GD * (g + 1)]).reshape(NE, 128, GD),
            "wkb": np.ascontiguousarray(
                wk[:, D * g:D * (g + 1)]).reshape(NE, 128, D),
            "wvb": np.ascontiguousarray(
                wv[:, D * g:D * (g + 1)]).reshape(NE, 128, D),
            "wob": np.ascontiguousarray(
                wo[GD * g:GD * (g + 1), :]).reshape(G, 128, E),
            "cosT": cosT,
            "sinT": sinT,
            "rotP": rotP,
            "ident": ident,
            "onesb": onesb,
        })

    res = run_bass_kernel_spmd(_NC, in_maps, list(range(NCORES)))
    LAST_RESULTS = res
    outs = [np.asarray(res.results[i]["out"], dtype=np.float32)
            for i in range(NCORES)]
    full = np.stack([sum(outs[b * G:(b + 1) * G]) for b in range(B)], axis=0)
    return full.astype(np.float32)


# revision 29
# speedup vs baseline: 1.1341x; 1.0138x over previous
"""GQA (16 q-heads / 4 kv-heads, D=128, S=2048, E=2048, B=2) on 8 trn2 cores.

Sharding: core = 4*b + g  (b in {0,1} batch, g in {0..3} kv-head group).
Each core computes its batch's 4 query heads (one kv group) end-to-end and
the host sums the 4 partial o_proj outputs per batch.

v3 (single interleaved program, engines balanced, DMA-consolidated):
  - Blocked DRAM layouts + AP.transpose give one DMA per weight tensor and
    4 DMAs per x position-chunk (~65 DMAs total vs ~250).
  - Phase A per 512-position chunk: K proj+RoPE, Q(h0) proj+RoPE, V proj +
    PE-transpose to natural bf16, Q(h1..h3) proj+RoPE.  RoPE rotate-half is
    a signed-permutation matmul on PE (no SBUF swap DMAs); cos/sin mults
    split DVE/DVE, add on Pool.
  - Attention in four 512-wide query chunks: scoresT per sk-tile in f32r,
    Exp on Act straight from PSUM to bf16 SBUF, bf16 AV matmuls (V-natural
    stationary), softmax denominator via two bf16 chain-adds (DVE + Pool),
    ones-matmul partition reduce, reciprocal, Pool partition_broadcast,
    DVE normalize multiply into f32r ot.
  - o_proj of chunk c interleaved into attention of chunk c+1; stores
    staged through SBUF [128,1024] tiles via DVE/Act copies.
"""

import numpy as np
import ml_dtypes

import concourse.bass as bass
import concourse.bacc as bacc
import concourse.mybir as mybir
import concourse.tile as tile
from concourse.bass_utils import run_bass_kernel_spmd

B, S, E = 2, 2048, 2048
H, HKV, D = 16, 4, 128
G = H // HKV          # 4 query heads per kv group
GD = G * D            # 512 channels per group
NCORES = 8
SCALE = 1.0 / float(np.sqrt(D))
ROPE_BASE = 10000.0

NE = E // 128         # 16 e-blocks (contraction for projections)
NC4 = S // 512        # 4 position chunks of 512 (projection granularity)
NST = S // 128        # 16 sk-tiles of 128
CHS = [512, 1024, 512]          # attention query-chunk widths
COFF = [0, 512, 1536]           # their offsets
CHMAX = 1024

F32 = mybir.dt.float32
F32R = mybir.dt.float32r
BF16 = mybir.dt.bfloat16
AF = mybir.ActivationFunctionType
OP = mybir.AluOpType


def _r(ap):
    return ap.bitcast(F32R)


def _emit(nc, tc, xTb, wqb, wkb, wvb, wob, cosT, sinT, rotP, ident, onesb, out):
    from contextlib import ExitStack
    es = ExitStack()
    with es:
        cpool = es.enter_context(tc.tile_pool(name="const", bufs=1))
        wopool = es.enter_context(tc.tile_pool(name="wo", bufs=2))
        xpool = es.enter_context(tc.tile_pool(name="xs", bufs=4))
        rpool = es.enter_context(tc.tile_pool(name="rope", bufs=2))
        etpool = es.enter_context(tc.tile_pool(name="et", bufs=6))
        bcspool = es.enter_context(tc.tile_pool(name="bcs", bufs=1))
        dnpool = es.enter_context(tc.tile_pool(name="dn", bufs=2))
        rcpool = es.enter_context(tc.tile_pool(name="rc", bufs=2))
        otpool = es.enter_context(tc.tile_pool(name="ot", bufs=6))
        ostgpool = es.enter_context(tc.tile_pool(name="ostg", bufs=3))
        vtpool = es.enter_context(tc.tile_pool(name="vt", bufs=2))
        pssc = es.enter_context(
            tc.tile_pool(name="pssc", bufs=2, space=bass.MemorySpace.PSUM))
        psav = es.enter_context(
            tc.tile_pool(name="psav", bufs=1, space=bass.MemorySpace.PSUM))
        psmx = es.enter_context(
            tc.tile_pool(name="psmx", bufs=2, space=bass.MemorySpace.PSUM))

        # ---- small constants ----
        id_sb = cpool.tile([128, 128], F32, tag="id")
        ones_sb = cpool.tile([128, 128], BF16, tag="ones")
        rp_sb = cpool.tile([128, 128], F32R, tag="rp")
        nc.sync.dma_start(out=id_sb[:], in_=ident.ap())
        nc.sync.dma_start(out=ones_sb[:], in_=onesb.ap())
        nc.sync.dma_start(out=rp_sb[:], in_=rotP.ap().bitcast(F32R))
        cos_sb = cpool.tile([D, S], BF16, tag="cos")
        sin_sb = cpool.tile([D, S], BF16, tag="sin")
        nc.sync.dma_start(out=cos_sb[:], in_=cosT.ap())
        nc.sync.dma_start(out=sin_sb[:], in_=sinT.ap())

        # ---- consolidated weight loads; wq/wv DMAs are issued after the
        # first x chunk so the critical path to the first K matmul is short
        wk_t = cpool.tile([128, NE, D], F32R, tag="wkt")
        nc.sync.dma_start(out=wk_t[:],
                          in_=wkb.ap().transpose([1, 0, 2]).bitcast(F32R))
        wq_t = cpool.tile([128, NE, GD], F32R, tag="wqt")
        wv_t = cpool.tile([128, NE, D], F32R, tag="wvt")

        # ---- persistent activations (bf16: same PE rate, half SBUF) ----
        kt = cpool.tile([D, S], BF16, tag="kt")
        qt = [cpool.tile([D, S], BF16, tag=f"qt{h}", name=f"qt{h}")
              for h in range(G)]
        vn = cpool.tile([128, NST, D], BF16, tag="vn")

        def rope(dst, ps, sl):
            # rotate_half as a signed-permutation matmul; then
            # dst = q*cos + rot(q)*sin.  The rot output borrows a scores-pool
            # PSUM slot (idle during phase A) to keep psmx free for the next
            # projection.
            qraw = rpool.tile([128, 512], F32R, tag="qraw")
            tmc = rpool.tile([128, 512], F32, tag="tmc")
            t2 = rpool.tile([128, 512], F32, tag="t2")
            nc.scalar.copy(qraw[:], ps[:])
            rot = pssc.tile([128, 512], F32, tag="sc", name="rot")
            nc.tensor.matmul(rot[:], rp_sb[:], qraw[:], start=True, stop=True)
            nc.vector.tensor_tensor(tmc[:], qraw[:], cos_sb[:, sl], OP.mult)
            nc.vector.tensor_tensor(t2[:], rot[:], sin_sb[:, sl], OP.mult)
            nc.gpsimd.tensor_tensor(dst, tmc[:], t2[:], OP.add)

        def proj(wt, cslice, xsl, c4):
            ps = psmx.tile([128, 512], F32, tag="mx", name="ps")
            for j in range(NE):
                nc.tensor.matmul(ps[:], wt[:, j, cslice],
                                 xsl[j // 4][:, j % 4, :],
                                 start=(j == 0), stop=(j == NE - 1))
            return ps

        # ================= phase A: projections + RoPE =================
        for c4 in range(NC4):
            sl = slice(c4 * 512, (c4 + 1) * 512)
            xsl = [xpool.tile([128, 4, 512], F32R, tag="xs",
                              name=f"xs{c4}_{jj}") for jj in range(4)]
            for jj in range(4):
                nc.sync.dma_start(
                    out=xsl[jj][:],
                    in_=xTb.ap()[jj * 4:(jj + 1) * 4, :, sl]
                        .transpose([1, 0, 2]).bitcast(F32R))
            if c4 == 0:
                for jj in range(4):
                    nc.sync.dma_start(
                        out=wq_t[:, jj * 4:(jj + 1) * 4, :],
                        in_=wqb.ap()[jj * 4:(jj + 1) * 4, :, :]
                            .transpose([1, 0, 2]).bitcast(F32R))
                nc.sync.dma_start(
                    out=wv_t[:],
                    in_=wvb.ap().transpose([1, 0, 2]).bitcast(F32R))
            # K
            ps = proj(wk_t, slice(0, D), xsl, c4)
            rope(kt[:, sl], ps, sl)
            # Q head 0 early so attention can start right after phase A
            ps = proj(wq_t, slice(0, D), xsl, c4)
            rope(qt[0][:, sl], ps, sl)
            # V -> natural bf16 via PE transpose
            ps = proj(wv_t, slice(0, D), xsl, c4)
            vt = vtpool.tile([D, 512], F32, tag="vt")
            nc.scalar.copy(vt[:], ps[:])
            for tt in range(4):
                trp = psmx.tile([128, 128], F32, tag="mx", name="trp")
                nc.tensor.transpose(trp[:], vt[:, tt * 128:(tt + 1) * 128],
                                    id_sb[:])
                nc.vector.tensor_copy(vn[:, c4 * 4 + tt, :], trp[:])
            # Q heads 1..3
            for h in range(1, G):
                ps = proj(wq_t, slice(h * D, (h + 1) * D), xsl, c4)
                rope(qt[h][:, sl], ps, sl)

        # ================= phases B+C: attention + o_proj =================
        def attn_head(off, chw, h):
            """Scores/exp/AV/denominator/normalize for a chw-wide query chunk
            at offset off, head h.  Returns the normalized [D, chw] f32r ot."""
            nhf = chw // 512
            dn = dnpool.tile([128, chw], BF16, tag="dn")
            av = psav.tile([D, chw], F32, tag="av")
            et_first = None
            for t in range(NST):
                sc = pssc.tile([128, chw], F32, tag="sc")
                for hf in range(nhf):
                    qsl = slice(off + hf * 512, off + (hf + 1) * 512)
                    nc.tensor.matmul(sc[:, hf * 512:(hf + 1) * 512],
                                     kt[:, t * 128:(t + 1) * 128],
                                     qt[h][:, qsl], start=True, stop=True)
                et = etpool.tile([128, chw], BF16, tag="et")
                nc.scalar.activation(et[:], sc[:], AF.Exp, scale=SCALE)
                for hf in range(nhf):
                    nc.tensor.matmul(av[:, hf * 512:(hf + 1) * 512],
                                     vn[:, t, :],
                                     et[:, hf * 512:(hf + 1) * 512],
                                     start=(t == 0), stop=(t == NST - 1))
                # denominator: single DVE chain, always caught up with exp
                if t == 0:
                    et_first = et
                elif t == 1:
                    nc.vector.tensor_tensor(dn[:], et_first[:], et[:], OP.add)
                else:
                    nc.vector.tensor_tensor(dn[:], dn[:], et[:], OP.add)
            rc = rcpool.tile([1, chw], BF16, tag="rc")
            for hf in range(nhf):
                sm = psmx.tile([1, 512], F32, tag="mx", name="sm")
                nc.tensor.matmul(sm[:], ones_sb[:, 0:1],
                                 dn[:, hf * 512:(hf + 1) * 512],
                                 start=True, stop=True)
                with nc.allow_low_precision(reason="bf16 softmax denom recip"):
                    nc.vector.reciprocal(rc[:, hf * 512:(hf + 1) * 512], sm[:])
            bcs = bcspool.tile([128, chw], BF16, tag="bcs")
            nc.gpsimd.partition_broadcast(bcs[:], rc[:])
            ot = otpool.tile([D, chw], F32R, tag="ot")
            nc.vector.tensor_tensor(ot[:], av[:], bcs[:], OP.mult)
            return ot

        def oproj_pair(off, chw, eo0, ots, ci):
            """Two adjacent eo column-groups (1024 cols of E) of o_proj for
            the chunk at offset off: one [128,1024] staging tile per s-tile,
            one store each."""
            wots = []
            for eo in (eo0, eo0 + 1):
                wot = wopool.tile([128, G, 512], F32R, tag="wo",
                                  name=f"wo{ci}_{eo}")
                nc.sync.dma_start(
                    out=wot[:],
                    in_=wob.ap()[:, :, eo * 512:(eo + 1) * 512]
                        .transpose([1, 0, 2]).bitcast(F32R))
                wots.append(wot)
            for st in range(chw // 128):
                ostg = ostgpool.tile([128, 1024], F32, tag="ostg",
                                     name=f"ostg{ci}_{eo0}_{st}")
                for k in range(2):
                    op = psmx.tile([128, 512], F32, tag="mx", name="op")
                    for h in range(G):
                        nc.tensor.matmul(op[:],
                                         ots[h][:, st * 128:(st + 1) * 128],
                                         wots[k][:, h, :],
                                         start=(h == 0), stop=(h == G - 1))
                    nc.vector.tensor_copy(ostg[:, k * 512:(k + 1) * 512], op[:])
                nc.sync.dma_start(
                    out=out.ap()[off + st * 128:off + (st + 1) * 128,
                                 eo0 * 512:(eo0 + 2) * 512],
                    in_=ostg[:])

        prev = None
        for ci, (off, chw) in enumerate(zip(COFF, CHS)):
            cur_ots = []
            for h in range(G):
                cur_ots.append(attn_head(off, chw, h))
                # interleave previous chunk's o_proj; front-load (one eo-pair
                # after each of heads 0/1) so at most 4 prev + 2 cur ot tiles
                # are ever live.
                if prev is not None and h < 2:
                    oproj_pair(prev[0], prev[1], 2 * h, prev[2], ci - 1)
            prev = (off, chw, cur_ots)
        oproj_pair(prev[0], prev[1], 0, prev[2], len(CHS) - 1)
        oproj_pair(prev[0], prev[1], 2, prev[2], len(CHS) - 1)


def _build():
    nc = bacc.Bacc("TRN2", target_bir_lowering=False, debug=False,
                   num_devices=NCORES)
    xTb = nc.dram_tensor("xTb", [NE, 128, S], F32, kind="ExternalInput")
    wqb = nc.dram_tensor("wqb", [NE, 128, GD], F32, kind="ExternalInput")
    wkb = nc.dram_tensor("wkb", [NE, 128, D], F32, kind="ExternalInput")
    wvb = nc.dram_tensor("wvb", [NE, 128, D], F32, kind="ExternalInput")
    wob = nc.dram_tensor("wob", [G, 128, E], F32, kind="ExternalInput")
    cosT = nc.dram_tensor("cosT", [D, S], BF16, kind="ExternalInput")
    sinT = nc.dram_tensor("sinT", [D, S], BF16, kind="ExternalInput")
    rotP = nc.dram_tensor("rotP", [128, 128], F32, kind="ExternalInput")
    ident = nc.dram_tensor("ident", [128, 128], F32, kind="ExternalInput")
    onesb = nc.dram_tensor("onesb", [128, 128], BF16, kind="ExternalInput")
    out = nc.dram_tensor("out", [S, E], F32, kind="ExternalOutput")
    with tile.TileContext(nc) as tc:
        _emit(nc, tc, xTb, wqb, wkb, wvb, wob, cosT, sinT, rotP, ident,
              onesb, out)
    nc.compile()
    return nc


def _rope_tables():
    inv = 1.0 / (ROPE_BASE ** (np.arange(0, D, 2, dtype=np.float64) / D))
    t = np.arange(S, dtype=np.float64)
    freqs = t[:, None] * inv[None, :]                    # [S, D/2]
    emb = np.concatenate([freqs, freqs], axis=-1)        # [S, D]
    cosT = np.cos(emb).T.astype(ml_dtypes.bfloat16)      # [D, S]
    sinT = np.sin(emb).T.astype(ml_dtypes.bfloat16)
    return np.ascontiguousarray(cosT), np.ascontiguousarray(sinT)


def _rot_perm():
    # rot(q)[d] = -q[d+64] for d<64, +q[d-64] for d>=64, as a stationary
    # matmul operand: rot = P^T @ q with P[k, m] below.
    p = np.zeros((128, 128), dtype=np.float32)
    for d in range(64):
        p[d + 64, d] = -1.0
        p[d, d + 64] = 1.0
    return p


_NC = None
LAST_RESULTS = None


def kernel(hidden_states, wq, wk, wv, wo):
    global _NC, LAST_RESULTS
    if _NC is None:
        _NC = _build()
    cosT, sinT = _rope_tables()
    ident = np.eye(128, dtype=np.float32)
    onesb = np.ones((128, 128), dtype=ml_dtypes.bfloat16)
    rotP = _rot_perm()
    hs = np.asarray(hidden_states, dtype=np.float32)
    wq = np.asarray(wq, dtype=np.float32)
    wk = np.asarray(wk, dtype=np.float32)
    wv = np.asarray(wv, dtype=np.float32)
    wo = np.asarray(wo, dtype=np.float32)

    in_maps = []
    for core in range(NCORES):
        b, g = divmod(core, G)
        in_maps.append({
            "xTb": np.ascontiguousarray(hs[b].T).reshape(NE, 128, S),
            "wqb": np.ascontiguousarray(
                wq[:, GD * g:GD * (g + 1)]).reshape(NE, 128, GD),
            "wkb": np.ascontiguousarray(
                wk[:, D * g:D * (g + 1)]).reshape(NE, 128, D),
            "wvb": np.ascontiguousarray(
                wv[:, D * g:D * (g + 1)]).reshape(NE, 128, D),
            "wob": np.ascontiguousarray(
                wo[GD * g:GD * (g + 1), :]).reshape(G, 128, E),
            "cosT": cosT,
            "sinT": sinT,
            "rotP": rotP,
            "ident": ident,
            "onesb": onesb,
        })

    res = run_bass_kernel_spmd(_NC, in_maps, list(range(NCORES)))
    LAST_RESULTS = res
    outs = [np.asarray(res.results[i]["out"], dtype=np.float32)
            for i in range(NCORES)]
    full = np.stack([sum(outs[b * G:(b + 1) * G]) for b in range(B)], axis=0)
    return full.astype(np.float32)
